# revision 1
# baseline (speedup 1.0000x reference)
"""Trainium2 Bass kernel for a pairwise-distance cluster margin loss.

Math (matches the jax reference):
    sq_i   = ||x_i||^2
    dist2  = sq_i + sq_j - 2 * x_i . x_j          (4096 x 4096)
    dist   = sqrt(max(dist2, eps))
    mask   = targets_i == targets_j
    far_i  = max_{j in class(i)} dist_ij
    near_i = second smallest dist_ij over class(i)  (smallest is self)
    loss   = mean(relu(far - near))

Strategy: row-shard the 4096 rows over 8 NeuronCores (512 rows each).
Each core streams the full x^T through its PE to produce, per
[128 x 512] PSUM tile,
    psA = dist2 + C*mask     (fp8e4m3 DoubleRow chain for the x part +
                              one bf16 aug matmul carrying sq hi/lo and
                              C*onehot class rows)
    psB = 2C*mask - 2^31*diag  (1-2 cheap bf16 matmuls)
On-chip reductions then give
    rowmax(psA)        = C + far2
    rowmax(psB - psA)  = C - near2   (diag pushed to -2^31, excluded)
and the host only applies max-over-slabs / sqrt / relu / mean to the
small reduced stats. fp8 quantization of x adds ~2e-4 relative error to
the loss (validated against an fp64 host model).

Each core's rhs slab order is rotated so its diagonal block is always
program-slab 0 - the diag fixup matmul is only emitted there (SPMD-safe,
no per-slab zero matmuls).
"""

import numpy as np
import ml_dtypes

BF = ml_dtypes.bfloat16
F8 = ml_dtypes.float8_e4m3

N = 4096  # rows (points)
D = 2048  # feature dim
P = 128  # partitions
NCORES = 8
MB = N // NCORES  # 512 rows per core
KX = D // P  # 16 x-chunks of 128
NT = N // 512  # 8 column tiles of 512
MT = MB // P  # 4 row tiles of 128 per core
NCLS = 64

C = float(2.0**17)  # mask offset; > max dist2 (~8.2k), keeps fp32 resolution
DIAG = -float(2.0**31)  # diagonal push-out in psB

_compiled = None


def _build_nc():
    import concourse.mybir as mybir
    import concourse.tile as tile
    from concourse import bacc
    from concourse.bass import ts

    nc = bacc.Bacc("TRN2", target_bir_lowering=False)
    f32 = mybir.dt.float32
    bf16 = mybir.dt.bfloat16
    fp8 = mybir.dt.float8e4
    DR = mybir.MatmulPerfMode.DoubleRow

    rhs8_d = nc.dram_tensor("rhs8", [NT, P, KX, 512], fp8, kind="ExternalInput")
    rhsa_d = nc.dram_tensor("rhsa", [NT, P, 512], bf16, kind="ExternalInput")
    lhs8_d = nc.dram_tensor("lhs8", [P, KX, MB], fp8, kind="ExternalInput")
    lhsaa_d = nc.dram_tensor("lhsaa", [P, MB], bf16, kind="ExternalInput")
    lhsb_d = nc.dram_tensor("lhsb", [P, MB], bf16, kind="ExternalInput")
    eye_d = nc.dram_tensor("eye", [P, P], bf16, kind="ExternalInput")
    dmat_d = nc.dram_tensor("dmat", [P, MT, 512], bf16, kind="ExternalInput")
    res_d = nc.dram_tensor("res", [MT, 2, P, NT], f32, kind="ExternalOutput")

    X = mybir.AxisListType.X

    with tile.TileContext(nc) as tc:
        with (
            tc.tile_pool(name="singles", bufs=1) as singles,
            tc.tile_pool(name="rhsp", bufs=3) as rhsp,
            tc.tile_pool(name="rhap", bufs=2) as rhap,
            tc.tile_pool(name="psa", bufs=5, space="PSUM") as psa,
            tc.tile_pool(name="psb", bufs=3, space="PSUM") as psb,
            tc.tile_pool(name="sbb", bufs=3) as sbb,
            tc.tile_pool(name="gsc", bufs=3) as gsc,
        ):
            lhs8 = singles.tile([P, KX, MB], fp8)
            rhs0 = rhsp.tile([P, KX, 512], fp8, name="rhs0")
            lhsb = singles.tile([P, MB], bf16)
            # smallest deps first: psB-mask matmuls can start on these alone
            nc.sync.dma_start(out=lhsb, in_=lhsb_d[:, :])
            rha0 = rhap.tile([P, 512], bf16, name="rha0")
            nc.sync.dma_start(out=rha0, in_=rhsa_d[0])
            eye = singles.tile([P, P], bf16)
            nc.sync.dma_start(out=eye, in_=eye_d[:, :])
            dmat = singles.tile([P, MT, 512], bf16)
            nc.sync.dma_start(out=dmat, in_=dmat_d[:, :, :])
            # interleave so the first psA chain's deps land earliest
            nc.sync.dma_start(out=lhs8[:, 0:1, :], in_=lhs8_d[:, 0:1, :])
            nc.sync.dma_start(out=rhs0[:, 0:1, :], in_=rhs8_d[0, :, 0:1, :])
            nc.sync.dma_start(out=lhs8[:, 1:3, :], in_=lhs8_d[:, 1:3, :])
            nc.sync.dma_start(out=rhs0[:, 1:3, :], in_=rhs8_d[0, :, 1:3, :])
            nc.sync.dma_start(out=lhs8[:, 3:5, :], in_=lhs8_d[:, 3:5, :])
            nc.sync.dma_start(out=rhs0[:, 3:5, :], in_=rhs8_d[0, :, 3:5, :])
            nc.sync.dma_start(out=lhs8[:, 5:7, :], in_=lhs8_d[:, 5:7, :])
            nc.sync.dma_start(out=rhs0[:, 5:7, :], in_=rhs8_d[0, :, 5:7, :])
            nc.sync.dma_start(out=lhs8[:, 7:10, :], in_=lhs8_d[:, 7:10, :])
            nc.sync.dma_start(out=rhs0[:, 7:10, :], in_=rhs8_d[0, :, 7:10, :])
            nc.sync.dma_start(out=lhs8[:, 10:13, :], in_=lhs8_d[:, 10:13, :])
            nc.sync.dma_start(out=rhs0[:, 10:13, :], in_=rhs8_d[0, :, 10:13, :])
            nc.sync.dma_start(out=lhs8[:, 13:KX, :], in_=lhs8_d[:, 13:KX, :])
            nc.sync.dma_start(out=rhs0[:, 13:KX, :], in_=rhs8_d[0, :, 13:KX, :])
            lhsaa = singles.tile([P, MB], bf16)
            nc.sync.dma_start(out=lhsaa, in_=lhsaa_d[:, :])

            fstats = [
                singles.tile([P, NT], f32, tag=f"fs{m}", name=f"fs{m}")
                for m in range(MT)
            ]
            gstats = [
                singles.tile([P, NT], f32, tag=f"gs{m}", name=f"gs{m}")
                for m in range(MT)
            ]

            for s in range(NT):
                if s == 0:
                    rhs = rhs0
                    rha = rha0
                else:
                    rhs = rhsp.tile([P, KX, 512], fp8, tag="rhs0", name="rhsl")
                    nc.sync.dma_start(out=rhs[:, 0:8, :], in_=rhs8_d[s, :, 0:8, :])
                    nc.sync.dma_start(out=rhs[:, 8:KX, :], in_=rhs8_d[s, :, 8:KX, :])
                    rha = rhap.tile([P, 512], bf16, tag="rha0", name="rhal")
                    nc.sync.dma_start(out=rha, in_=rhsa_d[s])

                for mt in range(MT):
                    # psB first: its deps are tiny, keeps PE busy during the
                    # initial x-chunk DMA
                    b = psb.tile([P, 512], f32)
                    nc.tensor.matmul(
                        b, lhsb[:, ts(mt, P)], rha, start=True, stop=(s != 0)
                    )
                    if s == 0:
                        nc.tensor.matmul(
                            b, eye, dmat[:, mt, :], start=False, stop=True
                        )
                    a = psa.tile([P, 512], f32)
                    if s == 0 and mt == 0:
                        # solo chunk 0/15 (non-DR) only for the very first
                        # tile: the first matmul then needs just one 128KB
                        # DMA landed, at the price of one extra instruction
                        nc.tensor.matmul(
                            a, lhs8[:, 0, ts(mt, P)], rhs[:, 0, :],
                            start=True, stop=False,
                        )
                        for c in range(1, KX - 1, 2):
                            nc.tensor.matmul(
                                a,
                                lhs8[:, c : c + 2, ts(mt, P)],
                                rhs[:, c : c + 2, :],
                                start=False,
                                stop=False,
                                perf_mode=DR,
                            )
                        nc.tensor.matmul(
                            a, lhs8[:, KX - 1, ts(mt, P)], rhs[:, KX - 1, :],
                            start=False, stop=False,
                        )
                    else:
                        for c in range(0, KX, 2):
                            nc.tensor.matmul(
                                a,
                                lhs8[:, c : c + 2, ts(mt, P)],
                                rhs[:, c : c + 2, :],
                                start=(c == 0),
                                stop=False,
                                perf_mode=DR,
                            )
                    nc.tensor.matmul(
                        a, lhsaa[:, ts(mt, P)], rha, start=False, stop=True
                    )
                    bb = sbb.tile([P, 512], f32)
                    nc.scalar.copy(bb, b)
                    nc.vector.reduce_max(fstats[mt][:, s : s + 1], a, axis=X)
                    # tensor_tensor_reduce would fuse these two, but that
                    # raw-ISA op dies on this compile path (NRT exec error)
                    g = gsc.tile([P, 512], f32)
                    nc.vector.tensor_sub(g, bb, a)
                    nc.vector.reduce_max(gstats[mt][:, s : s + 1], g, axis=X)

            for mt in range(MT):
                nc.sync.dma_start(out=res_d[mt, 0], in_=fstats[mt])
                nc.sync.dma_start(out=res_d[mt, 1], in_=gstats[mt])

    nc.compile()
    return nc


def _prep_inputs(x, t):
    """Host-side encode of the operands (x parts fp8, aug rows bf16)."""
    x = np.asarray(x, np.float32)
    t = np.asarray(t).astype(np.int64)
    sq = np.sum(x.astype(np.float64) ** 2, axis=1)
    sqhi = sq.astype(BF)
    sqlo = (sq - sqhi.astype(np.float64)).astype(BF)

    ohT = np.zeros((NCLS, N), BF)
    ohT[t, np.arange(N)] = BF(1.0)

    # fp8 x parts
    R8 = np.ascontiguousarray((-2.0 * x).astype(F8).T).reshape(KX, P, N)
    rhs8_np = np.ascontiguousarray(R8.reshape(KX, P, NT, 512).transpose(2, 1, 0, 3))
    L8 = np.ascontiguousarray(x.astype(F8).T).reshape(KX, P, N)

    # bf16 aug chunk: [sq_hi ; sq_lo ; 1 ; 1 ; C*onehot ; 0...]
    RA = np.zeros((P, N), BF)
    RA[0] = sqhi
    RA[1] = sqlo
    RA[2] = BF(1.0)
    RA[3] = BF(1.0)
    RA[4 : 4 + NCLS] = (C * ohT.astype(np.float32)).astype(BF)
    rhsa_np = np.ascontiguousarray(RA.reshape(P, NT, 512).transpose(1, 0, 2))

    LAA = np.zeros((P, N), BF)  # psA aug lhs: [1 ; 1 ; sq_hi ; sq_lo ; onehot]
    LAA[0] = BF(1.0)
    LAA[1] = BF(1.0)
    LAA[2] = sqhi
    LAA[3] = sqlo
    LAA[4 : 4 + NCLS] = ohT

    LB = np.zeros((P, N), BF)  # psB aug lhs: [0;0;0;0; 2*onehot]
    LB[4 : 4 + NCLS] = (2.0 * ohT.astype(np.float32)).astype(BF)

    eye_np = np.zeros((P, P), BF)
    eye_np[np.arange(P), np.arange(P)] = BF(1.0)

    dmat = np.zeros((P, MT, 512), BF)
    for mt in range(MT):
        dmat[np.arange(P), mt, mt * P + np.arange(P)] = BF(DIAG)

    in_maps = []
    for c0 in range(NCORES):
        sl = slice(c0 * MB, (c0 + 1) * MB)
        l8 = np.ascontiguousarray(L8[:, :, sl].transpose(1, 0, 2))  # [P, KX, MB]
        laa = np.ascontiguousarray(LAA[:, sl])
        lb = np.ascontiguousarray(LB[:, sl])
        # rotate slabs: program slab s holds global tile (c0 + s) % NT, so
        # the diagonal block is always at program slab 0
        r8 = np.ascontiguousarray(np.roll(rhs8_np, -c0, axis=0))
        ra = np.ascontiguousarray(np.roll(rhsa_np, -c0, axis=0))
        in_maps.append(
            {
                "rhs8": r8,
                "rhsa": ra,
                "lhs8": l8,
                "lhsaa": laa,
                "lhsb": lb,
                "eye": eye_np,
                "dmat": dmat,
            }
        )
    return in_maps


def _assemble(results):
    far2 = np.empty(N, np.float64)
    near2 = np.empty(N, np.float64)
    for c0 in range(NCORES):
        r = np.asarray(results[c0]["res"], np.float64)  # [MT, 2, P, NT]
        fmax = r[:, 0].max(axis=2)  # [MT, P]
        gmax = r[:, 1].max(axis=2)
        for mt in range(MT):
            idx = c0 * MB + mt * P + np.arange(P)
            far2[idx] = fmax[mt] - C
            near2[idx] = C - gmax[mt]
    far = np.sqrt(np.maximum(far2, 0.0))
    near = np.sqrt(np.maximum(near2, 0.0))
    loss = np.float32(np.mean(np.maximum(far - near, 0.0)))
    return np.asarray(loss, np.float32)


def run_kernel(inputs, targets, trace=False):
    """Returns (loss, BassKernelResults)."""
    from concourse.bass_utils import run_bass_kernel_spmd

    global _compiled
    if _compiled is None:
        _compiled = _build_nc()
    nc = _compiled
    in_maps = _prep_inputs(inputs, targets)
    br = run_bass_kernel_spmd(
        nc, in_maps, core_ids=list(range(NCORES)), trace=trace
    )
    return _assemble(br.results), br


def kernel(inputs, targets):
    loss, _ = run_kernel(inputs, targets)
    return loss



# revision 6
# speedup vs baseline: 2.9590x; 2.9590x over previous
"""Trainium2 Bass kernel for a pairwise-distance cluster margin loss.

Key observation: the loss only ever reads SAME-CLASS distances (the mask
selects targets_i == targets_j for both the farthest-positive and the
second-nearest-positive), so the full 4096x4096 distance matrix is
unnecessary. After grouping points by class on the host, only the 64
class-diagonal Gram blocks (~82x82 max, padded to 96) are needed:
~64x less matmul work than the full GEMM.

Per class c (padded to S=96 points, zero-padded cols/rows):
    G    = x_c @ x_c.T                       (fp8 DoubleRow chain, PSUM f32)
    A    = G - sq_v/2 (bf16 hi/lo aug rows; pad cols get +2^20)
    fmin = rowmin(A)      -> far2_u  = sq_u - 2*fmin   (pad cols excluded
                             by +2^20; diag is +sq_u/2, never the min)
    M    = A + bmat        (bmat: diag -2^20, pad cols -2^21, via eye matmul)
    gmax = rowmax(M)      -> near2_u = sq_u - 2*gmax   (diag+pad pushed out)
Host: far=sqrt(max(far2,1e-12)), near=sqrt(max(near2,1e-12)),
loss = mean(relu(far-near)).  8 classes per core, 8 cores.
"""

import numpy as np
import ml_dtypes

BF = ml_dtypes.bfloat16
F8 = ml_dtypes.float8_e4m3

N = 4096
D = 2048
P = 128
NCORES = 8
NCLS = 64
S = 96  # padded class size (max observed 82)
CPC = NCLS // NCORES  # 8 classes per core
KX = D // P  # 16 k-chunks of 128
HUGE = float(2.0**20)

_compiled = None


def _build_nc():
    import concourse.mybir as mybir
    import concourse.tile as tile
    from concourse import bacc

    nc = bacc.Bacc("TRN2", target_bir_lowering=False)
    f32 = mybir.dt.float32
    bf16 = mybir.dt.bfloat16
    fp8 = mybir.dt.float8e4
    DR = mybir.MatmulPerfMode.DoubleRow
    X = mybir.AxisListType.X

    xt8_d = nc.dram_tensor("xt8", [P, CPC, KX, S], fp8, kind="ExternalInput")
    faug_d = nc.dram_tensor("faug", [2, CPC, S], bf16, kind="ExternalInput")
    ones2_d = nc.dram_tensor("ones2", [2, S], bf16, kind="ExternalInput")
    bmat_d = nc.dram_tensor("bmat", [S, CPC, S], f32, kind="ExternalInput")
    res_d = nc.dram_tensor("res", [2, S, CPC], f32, kind="ExternalOutput")

    with tile.TileContext(nc) as tc:
        with (
            tc.tile_pool(name="singles", bufs=1) as singles,
            tc.tile_pool(name="scr", bufs=2) as scr,
            tc.tile_pool(name="ps", bufs=8, space="PSUM") as psp,
        ):
            xt8 = singles.tile([P, CPC, KX, S], fp8)
            nc.sync.dma_start(out=xt8[:, 0:1, :, :], in_=xt8_d[:, 0:1, :, :])
            ones2 = singles.tile([2, S], bf16)
            nc.sync.dma_start(out=ones2, in_=ones2_d[:, :])
            faug = singles.tile([2, CPC, S], bf16)
            nc.sync.dma_start(out=faug, in_=faug_d[:, :, :])
            for s in range(1, CPC):
                nc.sync.dma_start(
                    out=xt8[:, s : s + 1, :, :], in_=xt8_d[:, s : s + 1, :, :]
                )
            bmat = singles.tile([S, CPC, S], f32)
            nc.sync.dma_start(out=bmat, in_=bmat_d[:, :, :])

            fst = singles.tile([S, CPC], f32, name="fst")
            gst = singles.tile([S, CPC], f32, name="gst")

            for s in range(CPC):
                ps = psp.tile([S, S], f32)
                for c in range(0, KX, 2):
                    nc.tensor.matmul(
                        ps,
                        xt8[:, s, c : c + 2, :],
                        xt8[:, s, c : c + 2, :],
                        start=(c == 0),
                        stop=False,
                        perf_mode=DR,
                    )
                nc.tensor.matmul(
                    ps, ones2, faug[:, s, :], start=False, stop=True
                )
                nc.vector.tensor_reduce(
                    fst[:, s : s + 1], ps, axis=X, op=mybir.AluOpType.min
                )
                m = scr.tile([S, S], f32)
                nc.vector.tensor_add(m, ps, bmat[:, s, :])
                nc.vector.reduce_max(gst[:, s : s + 1], m, axis=X)

            nc.sync.dma_start(out=res_d[0], in_=fst)
            nc.sync.dma_start(out=res_d[1], in_=gst)

    nc.compile()
    return nc


def _prep_inputs(x, t):
    x = np.asarray(x, np.float32)
    t = np.asarray(t).astype(np.int64)
    sq = np.sum(x.astype(np.float64) ** 2, axis=1)

    order = np.argsort(t, kind="stable")
    sizes = np.bincount(t, minlength=NCLS)
    assert sizes.max() <= S, f"class size {sizes.max()} exceeds padding {S}"
    offs = np.zeros(NCLS + 1, np.int64)
    offs[1:] = np.cumsum(sizes)

    x8 = x.astype(F8)
    sqhalf = sq / 2.0
    hi = sqhalf.astype(BF)
    lo = (sqhalf - hi.astype(np.float64)).astype(BF)

    ones2_np = np.ones((2, S), BF)

    in_maps = []
    meta = []
    for core in range(NCORES):
        xt8_np = np.zeros((P, CPC, KX, S), F8)
        faug_np = np.zeros((2, CPC, S), BF)
        bmat_np = np.zeros((S, CPC, S), np.float32)
        cmeta = []
        for s in range(CPC):
            c = core * CPC + s
            idx = order[offs[c] : offs[c + 1]]
            n = len(idx)
            cmeta.append(idx)
            if n > 0:
                # [n, D] -> [D, n] -> [KX, P, n] -> [P, KX, n]
                blk = np.ascontiguousarray(x8[idx].T).reshape(KX, P, n)
                xt8_np[:, s, :, :n] = blk.transpose(1, 0, 2)
                faug_np[0, s, :n] = -hi[idx]
                faug_np[1, s, :n] = -lo[idx]
            faug_np[0, s, n:] = BF(HUGE)
            bmat_np[np.arange(S), s, np.arange(S)] = -HUGE
            bmat_np[:n, s, n:] = -2 * HUGE
            bmat_np[n:, s, n:] += -2 * HUGE
        in_maps.append(
            {
                "xt8": xt8_np,
                "faug": faug_np,
                "ones2": ones2_np,
                "bmat": bmat_np,
            }
        )
        meta.append(cmeta)
    return in_maps, meta, sq


def _assemble(results, meta, sq):
    far2 = np.empty(N, np.float64)
    near2 = np.empty(N, np.float64)
    for core in range(NCORES):
        r = np.asarray(results[core]["res"], np.float64)  # [2, S, CPC]
        for s in range(CPC):
            idx = meta[core][s]
            n = len(idx)
            if n == 0:
                continue
            far2[idx] = sq[idx] - 2.0 * r[0, :n, s]
            near2[idx] = sq[idx] - 2.0 * r[1, :n, s]
    far = np.sqrt(np.maximum(far2, 1e-12))
    near = np.sqrt(np.maximum(near2, 1e-12))
    loss = np.float32(np.mean(np.maximum(far - near, 0.0)))
    return np.asarray(loss, np.float32)


def run_kernel(inputs, targets, trace=False):
    """Returns (loss, BassKernelResults)."""
    from concourse.bass_utils import run_bass_kernel_spmd

    global _compiled
    if _compiled is None:
        _compiled = _build_nc()
    nc = _compiled
    in_maps, meta, sq = _prep_inputs(inputs, targets)
    br = run_bass_kernel_spmd(
        nc, in_maps, core_ids=list(range(NCORES)), trace=trace
    )
    return _assemble(br.results, meta, sq), br


def kernel(inputs, targets):
    loss, _ = run_kernel(inputs, targets)
    return loss


# revision 9
# speedup vs baseline: 3.4317x; 1.1597x over previous
"""Trainium2 Bass kernel for a pairwise-distance cluster margin loss.

Key observation: the loss only ever reads SAME-CLASS distances (the mask
selects targets_i == targets_j for both the farthest-positive and the
second-nearest-positive), so the full 4096x4096 distance matrix is
unnecessary. After grouping points by class on the host, only the 64
class-diagonal Gram blocks (~82x82 max, padded to 96) are needed:
~64x less matmul work than the full GEMM.

Per class c (padded to S=96 points, zero-padded cols/rows):
    G    = x_c @ x_c.T                       (fp8 DoubleRow chain, PSUM f32)
    A    = G - sq_v/2 (bf16 hi/lo aug rows; pad cols get +2^20)
    fmin = rowmin(A)      -> far2_u  = sq_u - 2*fmin   (pad cols excluded
                             by +2^20; diag is +sq_u/2, never the min)
    M    = A + bmat        (bmat: diag -2^20, pad cols -2^21, via eye matmul)
    gmax = rowmax(M)      -> near2_u = sq_u - 2*gmax   (diag+pad pushed out)
Host: far=sqrt(max(far2,1e-12)), near=sqrt(max(near2,1e-12)),
loss = mean(relu(far-near)).  8 classes per core, 8 cores.
"""

import numpy as np
import ml_dtypes

BF = ml_dtypes.bfloat16
F8 = ml_dtypes.float8_e4m3

N = 4096
D = 2048
P = 128
NCORES = 8
NCLS = 64
S = 96  # padded class size (max observed 82)
CPC = NCLS // NCORES  # 8 classes per core
KX = D // P  # 16 k-chunks of 128
HUGE = float(2.0**20)

_compiled = None


def _build_nc():
    import concourse.mybir as mybir
    import concourse.tile as tile
    from concourse import bacc

    nc = bacc.Bacc("TRN2", target_bir_lowering=False)
    f32 = mybir.dt.float32
    bf16 = mybir.dt.bfloat16
    fp8 = mybir.dt.float8e4
    DR = mybir.MatmulPerfMode.DoubleRow
    X = mybir.AxisListType.X

    xt8_d = nc.dram_tensor("xt8", [P, CPC, KX, S], fp8, kind="ExternalInput")
    faug_d = nc.dram_tensor("faug", [2, CPC, S], bf16, kind="ExternalInput")
    ones2_d = nc.dram_tensor("ones2", [2, S], bf16, kind="ExternalInput")
    bmat_d = nc.dram_tensor("bmat", [S, CPC, S], f32, kind="ExternalInput")
    res_d = nc.dram_tensor("res", [S, 2, CPC], f32, kind="ExternalOutput")

    with tile.TileContext(nc) as tc:
        with (
            tc.tile_pool(name="singles", bufs=1) as singles,
            tc.tile_pool(name="scr", bufs=2) as scr,
            tc.tile_pool(name="ps", bufs=8, space="PSUM") as psp,
        ):
            # two HWDGE queues: scalar carries slot0 + tail slots, sync
            # carries the small tiles + bmat + middle slots
            xt8 = singles.tile([P, CPC, KX, S], fp8)
            nc.scalar.dma_start(out=xt8[:, 0:1, :, :], in_=xt8_d[:, 0:1, :, :])
            ones2 = singles.tile([2, S], bf16)
            nc.sync.dma_start(out=ones2, in_=ones2_d[:, :])
            faug = singles.tile([2, CPC, S], bf16)
            nc.sync.dma_start(out=faug, in_=faug_d[:, :, :])
            bmat = singles.tile([S, CPC, S], f32)
            nc.sync.dma_start(out=bmat, in_=bmat_d[:, :, :])
            nc.scalar.dma_start(
                out=xt8[:, 1:4, :, :], in_=xt8_d[:, 1:4, :, :]
            )
            nc.sync.dma_start(
                out=xt8[:, 4 : CPC, :, :], in_=xt8_d[:, 4:CPC, :, :]
            )

            res = singles.tile([S, 2, CPC], f32, name="res")
            fst = res[:, 0, :]
            gst = res[:, 1, :]

            for s in range(CPC):
                ps = psp.tile([S, S], f32, padded_shape=[S, 512])
                for c in range(0, KX, 2):
                    nc.tensor.matmul(
                        ps,
                        xt8[:, s, c : c + 2, :],
                        xt8[:, s, c : c + 2, :],
                        start=(c == 0),
                        stop=False,
                        perf_mode=DR,
                    )
                nc.tensor.matmul(
                    ps, ones2, faug[:, s, :], start=False, stop=True
                )
                nc.vector.tensor_reduce(
                    fst[:, s : s + 1], ps, axis=X, op=mybir.AluOpType.min
                )
                m = scr.tile([S, S], f32)
                nc.vector.tensor_add(m, ps, bmat[:, s, :])
                nc.vector.reduce_max(gst[:, s : s + 1], m, axis=X)

            nc.sync.dma_start(out=res_d[:, :, :], in_=res)

    nc.compile()
    return nc


def _prep_inputs(x, t):
    x = np.asarray(x, np.float32)
    t = np.asarray(t).astype(np.int64)
    sq = np.sum(x.astype(np.float64) ** 2, axis=1)

    order = np.argsort(t, kind="stable")
    sizes = np.bincount(t, minlength=NCLS)
    assert sizes.max() <= S, f"class size {sizes.max()} exceeds padding {S}"
    offs = np.zeros(NCLS + 1, np.int64)
    offs[1:] = np.cumsum(sizes)

    x8 = x.astype(F8)
    sqhalf = sq / 2.0
    hi = sqhalf.astype(BF)
    lo = (sqhalf - hi.astype(np.float64)).astype(BF)

    ones2_np = np.ones((2, S), BF)

    in_maps = []
    meta = []
    for core in range(NCORES):
        xt8_np = np.zeros((P, CPC, KX, S), F8)
        faug_np = np.zeros((2, CPC, S), BF)
        bmat_np = np.zeros((S, CPC, S), np.float32)
        cmeta = []
        for s in range(CPC):
            c = core * CPC + s
            idx = order[offs[c] : offs[c + 1]]
            n = len(idx)
            cmeta.append(idx)
            if n > 0:
                # [n, D] -> [D, n] -> [KX, P, n] -> [P, KX, n]
                blk = np.ascontiguousarray(x8[idx].T).reshape(KX, P, n)
                xt8_np[:, s, :, :n] = blk.transpose(1, 0, 2)
                faug_np[0, s, :n] = -hi[idx]
                faug_np[1, s, :n] = -lo[idx]
            faug_np[0, s, n:] = BF(HUGE)
            bmat_np[np.arange(S), s, np.arange(S)] = -HUGE
            bmat_np[:n, s, n:] = -2 * HUGE
            bmat_np[n:, s, n:] += -2 * HUGE
        in_maps.append(
            {
                "xt8": xt8_np,
                "faug": faug_np,
                "ones2": ones2_np,
                "bmat": bmat_np,
            }
        )
        meta.append(cmeta)
    return in_maps, meta, sq


def _assemble(results, meta, sq):
    far2 = np.empty(N, np.float64)
    near2 = np.empty(N, np.float64)
    for core in range(NCORES):
        r = np.asarray(results[core]["res"], np.float64)  # [S, 2, CPC]
        for s in range(CPC):
            idx = meta[core][s]
            n = len(idx)
            if n == 0:
                continue
            far2[idx] = sq[idx] - 2.0 * r[:n, 0, s]
            near2[idx] = sq[idx] - 2.0 * r[:n, 1, s]
    far = np.sqrt(np.maximum(far2, 1e-12))
    near = np.sqrt(np.maximum(near2, 1e-12))
    loss = np.float32(np.mean(np.maximum(far - near, 0.0)))
    return np.asarray(loss, np.float32)


def run_kernel(inputs, targets, trace=False):
    """Returns (loss, BassKernelResults)."""
    from concourse.bass_utils import run_bass_kernel_spmd

    global _compiled
    if _compiled is None:
        _compiled = _build_nc()
    nc = _compiled
    in_maps, meta, sq = _prep_inputs(inputs, targets)
    br = run_bass_kernel_spmd(
        nc, in_maps, core_ids=list(range(NCORES)), trace=trace
    )
    return _assemble(br.results, meta, sq), br


def kernel(inputs, targets):
    loss, _ = run_kernel(inputs, targets)
    return loss
